# revision 1
# baseline (speedup 1.0000x reference)
"""Trainium2 Bass kernel: DGCNN forward (4-layer GCN + Conv1d readout) on 8 NeuronCores.

Math restructuring (verified vs reference):
  With A = D^-1/2 (Adj + I) D^-1/2 and Mk / ck derived from the (tiny) weights,
    out = A(x M1 + A(x M2 + A(x M3 + A(x M4)))) + 1 c0 + v1 c1 + v2 c2 + v3 c3
  All four aggregation passes are width-16: the innermost table
  T3 = dinv*(x M4) is computed locally per core (PE matmul) and exchanged,
  so no 64-wide x-table pass is needed.

Device strategy (graph-parallel over 8 cores):
  - Nodes are permuted: degree-sorted, dealt into 128-row blocks round-robin
    across cores; same-index blocks across cores have near-equal max degree.
  - Self-loop messages are NOT gathered: each pass's epilogue adds the
    core-local previous-stage value (d2 * st_prev) instead, cutting ~6% of
    gather rows and 1 ELL slot per block.
  - Per pass: messages are fetched with dma_gather (256B rows) from a DRAM
    table into an ELL-padded [128, slots, 64] tile (dst j of block b -> SBUF
    partition j; only cols 0:16 are meaningful), then segment-summed with one
    strided DVE reduce per block.
  - int16 gather indices limit a window to 32768 rows; the 50176-row table is
    covered by two overlapping windows ([0,32768) and [17408,50176)), and each
    dst's edge list is split between the windows (balanced using the overlap),
    padded with a dedicated all-zero table row.
  - Gathers use prepare_only=True: descriptor generation (the Q7 bottleneck)
    is software-pipelined ~4 gather-units ahead of the trigger_dma that fires
    each DMA, so desc-gen runs across pass boundaries instead of stalling on
    the table exchange.
  - After each pass every core computes its own rows of the next table
    (T = dinv*(x Mk + S), via PE matmuls) and an 8-core AllGather rebuilds the
    full table in DRAM (split 12/4 groups so most of the collective overlaps
    the pass tail). The AllGather output is restrided into the table with a
    single DRAM->DRAM DMA per (core, half).
  - dinv pre/post scaling is folded into the table rows, so no per-edge
    normalization multiplies are needed.
"""

import dataclasses
import numpy as np

import concourse.bass as bass
import concourse.bacc as bacc
import concourse.tile as tile
from concourse import mybir
from concourse.bass_utils import run_bass_kernel_spmd

F32 = mybir.dt.float32
I16 = mybir.dt.int16
AF = mybir.ActivationFunctionType


@dataclasses.dataclass(frozen=True)
class Cfg:
    N: int = 50000          # real nodes
    F: int = 64             # features
    NCORES: int = 8
    P: int = 128
    NBLK: int = 49          # dst blocks per core
    NGRP: int = 16          # gather groups per pass
    H1GRP: int = 13         # groups in exchange half 1
    NQ: int = 4             # SWDGE queues
    LOOK: int = 4           # prep lookahead (gather units)
    # prepare_only + trigger_dma pipelining is left OFF: the uCode's
    # prepared-gather path fires DMAs that intermittently miss their SBUF
    # writes (verified with minimal standalone tests), so gathers run in
    # direct (auto-fire) mode.
    PIPELINE: bool = False
    SINGLE_PACKET: bool = False

    @property
    def PER(self):
        return self.NBLK * self.P

    @property
    def NPAD(self):
        return self.NCORES * self.PER

    @property
    def WA_LEN(self):
        return min(32768, self.NPAD)

    @property
    def WB_OFF(self):
        return self.NPAD - self.WA_LEN


CFG = Cfg()

# results of the last device run (for test harness profiling)
LAST_RESULTS = None


# --------------------------------------------------------------------------
# host preprocessing
# --------------------------------------------------------------------------

def _host_prep(inputs, cfg: Cfg):
    x = np.asarray(inputs["x"], np.float32)
    ei = np.asarray(inputs["edge_index"]).astype(np.int64)
    W = [np.asarray(inputs[f"W{i}"], np.float64) for i in range(4)]
    b = [np.asarray(inputs[f"b{i}"], np.float64) for i in range(4)]
    conv_w = np.asarray(inputs["conv_w"], np.float64)
    conv_b = np.asarray(inputs["conv_b"], np.float64)

    n = x.shape[0]
    assert n == cfg.N and x.shape[1] == cfg.F
    P, PER, NPAD, NBLK, NC = cfg.P, cfg.PER, cfg.NPAD, cfg.NBLK, cfg.NCORES

    # degrees INCLUDE the self-loop (as in the reference)
    src_l = np.concatenate([ei[0], np.arange(n, dtype=np.int64)])
    dst_l = np.concatenate([ei[1], np.arange(n, dtype=np.int64)])
    deg = np.bincount(dst_l, minlength=n).astype(np.float64)
    dinv = 1.0 / np.sqrt(np.maximum(deg, 1.0))

    # ---- weight-derived small matrices ----
    Cw = [conv_w[:, 0:64], conv_w[:, 64:128], conv_w[:, 128:192], conv_w[:, 192:193]]
    M1 = W[0] @ Cw[0].T
    M2 = W[0] @ W[1] @ Cw[1].T
    M3 = W[0] @ W[1] @ W[2] @ Cw[2].T
    M4 = W[0] @ W[1] @ W[2] @ W[3] @ Cw[3].T
    c0 = b[0] @ Cw[0].T + b[1] @ Cw[1].T + b[2] @ Cw[2].T + b[3] @ Cw[3].T + conv_b
    c1 = (b[0] @ W[1]) @ Cw[1].T + (b[1] @ W[2]) @ Cw[2].T + (b[2] @ W[3]) @ Cw[3].T
    c2 = (b[0] @ W[1] @ W[2]) @ Cw[2].T + (b[1] @ W[2] @ W[3]) @ Cw[3].T
    c3 = (b[0] @ W[1] @ W[2] @ W[3]) @ Cw[3].T

    def aggv(v):
        o = np.zeros(n)
        np.add.at(o, dst_l, (v * dinv)[src_l])
        return o * dinv

    v1 = aggv(np.ones(n))
    v2 = aggv(v1)
    v3 = aggv(v2)
    bias = (np.outer(np.ones(n), c0) + np.outer(v1, c1)
            + np.outer(v2, c2) + np.outer(v3, c3))  # [n, 16]

    # ---- permutation: degree sort, deal into blocks; group blocks by size ----
    order = np.argsort(-deg, kind="stable")
    order_p = np.concatenate([order, np.full(NPAD - n, -1, np.int64)])
    assert NPAD - n >= 2
    # force a dummy (all-zero row) into (core 0, class NBLK-1, j=P-1)
    rA = ((NBLK - 1) * NC + 0) * P + (P - 1)
    order_p[rA], order_p[NPAD - 1] = order_p[NPAD - 1], order_p[rA]

    # degree-rank class of each padded rank (class = which 128x8 stripe)
    cls_of_rank = np.arange(NPAD) // P // NC
    # class max NON-SELF degree for grouping estimate
    deg_p = np.zeros(NPAD)
    real_rank = order_p >= 0
    deg_p[real_rank] = deg[order_p[real_rank]] - 1.0
    mTc = np.zeros(NBLK)
    np.maximum.at(mTc, cls_of_rank, deg_p)

    # group classes into NGRP gather groups, balancing estimated slots
    per_grp = NBLK // cfg.NGRP + (1 if NBLK % cfg.NGRP else 0)
    groups_c = [[] for _ in range(cfg.NGRP)]
    gsum = np.zeros(cfg.NGRP)
    for bq in np.argsort(-mTc, kind="stable"):
        cand = sorted(range(cfg.NGRP),
                      key=lambda q: (len(groups_c[q]) >= per_grp, gsum[q], q))
        q = cand[0]
        groups_c[q].append(int(bq))
        gsum[q] += mTc[bq]
    # renumber classes -> local block ids, groups 0..H1GRP-1 first (contiguous
    # row range for the first-half collective)
    order_cls = [c for q in range(cfg.NGRP) for c in groups_c[q]]
    renum = np.zeros(NBLK, np.int64)
    for newid, c in enumerate(order_cls):
        renum[c] = newid
    nblk_h1 = sum(len(groups_c[q]) for q in range(cfg.H1GRP))
    groups = []
    pos = 0
    for q in range(cfg.NGRP):
        groups.append(list(range(pos, pos + len(groups_c[q]))))
        pos += len(groups_c[q])

    g = np.arange(NPAD) // P
    j = np.arange(NPAD) % P
    npos_of_rank = (g % NC) * PER + renum[g // NC] * P + j
    pos2old = np.full(NPAD, -1, np.int64)
    pos2old[npos_of_rank] = order_p
    old2new = np.full(n, -1, np.int64)
    rmask = pos2old >= 0
    old2new[pos2old[rmask]] = np.nonzero(rmask)[0]

    zA = int(renum[NBLK - 1]) * P + (P - 1)          # dummy row, core 0
    zB = (NC - 1) * PER + int(renum[NBLK - 1]) * P + (P - 2)  # dummy row, core NC-1
    assert pos2old[zA] < 0 and pos2old[zB] < 0
    assert zA < cfg.WA_LEN and zB >= cfg.WB_OFF

    # ---- per-edge window split (REAL edges only; self-loops handled locally)
    s_new = old2new[ei[0]]
    d_new = old2new[ei[1]]
    eo = np.argsort(d_new, kind="stable")
    s_s = s_new[eo]
    d_s = d_new[eo]
    starts = np.searchsorted(d_s, np.arange(NPAD + 1))

    isA = s_s < cfg.WB_OFF
    isB = s_s >= cfg.WA_LEN
    isF = ~(isA | isB)
    nAo = np.bincount(d_s, weights=isA, minlength=NPAD).astype(np.int64)
    nBo = np.bincount(d_s, weights=isB, minlength=NPAD).astype(np.int64)
    nf = np.bincount(d_s, weights=isF, minlength=NPAD).astype(np.int64)
    tot = nAo + nBo + nf

    blk_pos = (np.arange(NPAD) % PER) // P
    mT = np.zeros(NBLK, np.int64); np.maximum.at(mT, blk_pos, tot)
    mA = np.zeros(NBLK, np.int64); np.maximum.at(mA, blk_pos, nAo)
    mB = np.zeros(NBLK, np.int64); np.maximum.at(mB, blk_pos, nBo)
    M = np.maximum(mT, mA + mB)
    SA = np.clip((M + 1) // 2, mA, M - mB)
    SA = np.maximum(SA, 1)
    SB = M - SA
    SAp = SA[blk_pos]
    SBp = SB[blk_pos]
    nA = np.clip(tot - SBp, nAo, np.minimum(nAo + nf, SAp))

    cFex = np.concatenate([[0], np.cumsum(isF)])
    frank = cFex[:-1] - cFex[starts[d_s]]
    goA = isA | (isF & (frank < (nA - nAo)[d_s]))
    goB = ~goA
    cAex = np.concatenate([[0], np.cumsum(goA)])
    slotA = cAex[:-1] - cAex[starts[d_s]]
    cBex = np.concatenate([[0], np.cumsum(goB)])
    slotB = cBex[:-1] - cBex[starts[d_s]]
    nB = tot - nA
    assert (nA <= SAp).all() and (nB <= SBp).all()

    oa = np.zeros(NBLK, np.int64)
    ob = np.zeros(NBLK, np.int64)
    grp_of = np.zeros(NBLK, np.int64)
    SAg = np.zeros(cfg.NGRP, np.int64)
    SBg = np.zeros(cfg.NGRP, np.int64)
    for q, bl in enumerate(groups):
        offa = 0
        for bq in bl:
            oa[bq] = offa
            offa += SA[bq]
            grp_of[bq] = q
        offb = 0
        for bq in bl:
            ob[bq] = offb
            offb += SB[bq]
        SAg[q] = offa
        SBg[q] = offb

    colA0 = np.zeros(cfg.NGRP, np.int64)
    colB0 = np.zeros(cfg.NGRP, np.int64)
    cur = 0
    for q in range(cfg.NGRP):
        colA0[q] = cur
        cur += SAg[q] * P // 16
        colB0[q] = cur
        cur += SBg[q] * P // 16
    idxcols = int(cur)

    # ---- build per-core idx tensors ----
    zA_rel = np.int16(zA)
    zB_rel = np.int16(zB - cfg.WB_OFF)
    idx_np = np.empty((NC, 128, idxcols), np.int16)
    # defaults: zero-row padding everywhere
    for q in range(cfg.NGRP):
        idx_np[:, :, colA0[q]:colA0[q] + SAg[q] * P // 16] = zA_rel
        idx_np[:, :, colB0[q]:colB0[q] + SBg[q] * P // 16] = zB_rel

    e_core = d_s // PER
    e_blk = (d_s % PER) // P
    e_j = d_s % P
    e_q = grp_of[e_blk]
    # linear position within the group's gather + column in the idx tensor
    posA = (oa[e_blk] + slotA) * P + e_j
    colA = colA0[e_q] + posA // 16
    rowA = posA % 16
    posB = (ob[e_blk] + slotB) * P + e_j
    colB = colB0[e_q] + posB // 16
    rowB = posB % 16
    valA = s_s.astype(np.int16)                   # window A offset is 0
    valB = (s_s - cfg.WB_OFF).astype(np.int16)
    for k in range(NC):
        mk = e_core == k
        mAm = mk & goA
        mBm = mk & goB
        for r in range(8):
            idx_np[k, rowA[mAm] + 16 * r, colA[mAm]] = valA[mAm]
            idx_np[k, rowB[mBm] + 16 * r, colB[mBm]] = valB[mBm]

    # ---- dense per-core arrays ----
    x_perm = np.zeros((NPAD, cfg.F), np.float32)
    x_perm[rmask] = x[pos2old[rmask]]
    dinv_perm = np.ones(NPAD, np.float32)
    dinv_perm[rmask] = dinv[pos2old[rmask]].astype(np.float32)
    bias_perm = np.zeros((NPAD, 16), np.float32)
    bias_perm[rmask] = bias[pos2old[rmask]].astype(np.float32)

    xT = [np.ascontiguousarray(x_perm[k * PER:(k + 1) * PER].T) for k in range(NC)]
    dinv_blk = [np.ascontiguousarray(dinv_perm[k * PER:(k + 1) * PER].reshape(NBLK, P).T)
                for k in range(NC)]
    dinv2_blk = [d * d for d in dinv_blk]
    bias_blk = [np.ascontiguousarray(
        bias_perm[k * PER:(k + 1) * PER].reshape(NBLK, P, 16).transpose(1, 0, 2))
        for k in range(NC)]
    mmats = np.ascontiguousarray(np.concatenate([M3, M2, M1], axis=1).astype(np.float32))
    m4 = np.ascontiguousarray(M4.astype(np.float32))

    layout = dict(SA=SA, SB=SB, groups=groups, oa=oa, ob=ob, SAg=SAg, SBg=SBg,
                  colA0=colA0, colB0=colB0, idxcols=idxcols, nblk_h1=nblk_h1)
    in_maps = []
    for k in range(NC):
        in_maps.append(dict(
            idx=np.ascontiguousarray(idx_np[k]),
            xT=xT[k],
            dinv_blk=dinv_blk[k],
            dinv2_blk=dinv2_blk[k],
            bias_blk=bias_blk[k],
            mmats=mmats,
            m4=m4,
        ))
    return in_maps, layout, old2new


# --------------------------------------------------------------------------
# numpy emulation of the device algorithm (for offline validation)
# --------------------------------------------------------------------------

def _algo_sim(in_maps, layout, cfg: Cfg):
    """Emulate the device algorithm exactly (decoding idx tensors)."""
    P, PER, NPAD, NBLK, NC = cfg.P, cfg.PER, cfg.NPAD, cfg.NBLK, cfg.NCORES
    SA, SB = layout["SA"], layout["SB"]
    groups, oa, ob = layout["groups"], layout["oa"], layout["ob"]
    SAg, SBg = layout["SAg"], layout["SBg"]
    colA0, colB0 = layout["colA0"], layout["colB0"]

    mm = in_maps[0]["mmats"]  # [64, 48] = M3|M2|M1
    m4 = in_maps[0]["m4"]

    # per-core stage values [NC][P, NBLK, 16]
    def stage_T3():
        sts = []
        for k in range(NC):
            xTk = in_maps[k]["xT"]     # [64, PER]
            db = in_maps[k]["dinv_blk"]  # [P, NBLK]
            st = np.zeros((P, NBLK, 16), np.float32)
            for bq in range(NBLK):
                xb = xTk[:, bq * P:(bq + 1) * P].T       # [128, 64]
                st[:, bq, :] = (xb @ m4) * db[:, bq:bq + 1]
            sts.append(st)
        return sts

    def build_table(sts):
        tab = np.zeros((NPAD, cfg.F), np.float32)
        for k in range(NC):
            tab[k * PER:(k + 1) * PER, 0:16] = (
                sts[k].transpose(1, 0, 2).reshape(PER, 16))
        return tab

    def run_pass(tab, sts_prev, p):
        winA = tab[0:cfg.WA_LEN]
        winB = tab[cfg.WB_OFF:NPAD]
        sts_new = []
        for k in range(NC):
            idx = in_maps[k]["idx"]
            db = in_maps[k]["dinv_blk"]
            d2 = in_maps[k]["dinv2_blk"]
            xTk = in_maps[k]["xT"]
            st_new = np.zeros((P, NBLK, 16), np.float32)
            for q, bl in enumerate(groups):
                sag, sbg = int(SAg[q]), int(SBg[q])
                # decode gathers
                colsA = idx[0:16, colA0[q]:colA0[q] + sag * P // 16]
                flatA = colsA.T.reshape(-1)  # pos = col*16 + row
                gA = winA[flatA].reshape(sag, P, cfg.F).transpose(1, 0, 2)
                if sbg:
                    colsB = idx[0:16, colB0[q]:colB0[q] + sbg * P // 16]
                    flatB = colsB.T.reshape(-1)
                    gB = winB[flatB].reshape(sbg, P, cfg.F).transpose(1, 0, 2)
                for bq in bl:
                    a0, a1 = int(oa[bq]), int(oa[bq] + SA[bq])
                    R = gA[:, a0:a1, 0:16].sum(axis=1)
                    if SB[bq]:
                        b0_, b1_ = int(ob[bq]), int(ob[bq] + SB[bq])
                        R = R + gB[:, b0_:b1_, 0:16].sum(axis=1)
                    Rf = R + sts_prev[k][:, bq, :]
                    if p < 3:
                        xb = xTk[:, bq * P:(bq + 1) * P].T
                        ps = xb @ mm[:, 16 * p:16 * p + 16]
                        st_new[:, bq, :] = (ps * db[:, bq:bq + 1]
                                            + Rf * d2[:, bq:bq + 1])
                    else:
                        st_new[:, bq, :] = (Rf * db[:, bq:bq + 1]
                                            + in_maps[k]["bias_blk"][:, bq, :])
            sts_new.append(st_new)
        return sts_new

    sts = stage_T3()
    for p in range(4):
        tab = build_table(sts)
        sts = run_pass(tab, sts, p)
    return sts  # final output stages


# --------------------------------------------------------------------------
# device module
# --------------------------------------------------------------------------

def _build_module(cfg: Cfg, layout):
    P, PER, NPAD, NBLK, NC = cfg.P, cfg.PER, cfg.NPAD, cfg.NBLK, cfg.NCORES
    SA, SB = layout["SA"], layout["SB"]
    groups, oa, ob = layout["groups"], layout["oa"], layout["ob"]
    SAg, SBg = layout["SAg"], layout["SBg"]
    colA0, colB0 = layout["colA0"], layout["colB0"]
    idxcols = layout["idxcols"]
    nblk_h1 = layout["nblk_h1"]
    nblk_h2 = NBLK - nblk_h1
    rows1, rows2 = nblk_h1 * P, nblk_h2 * P

    nc = bacc.Bacc("TRN2", target_bir_lowering=False, debug=False, num_devices=NC,
                   num_swdge_queues=cfg.NQ, dynamic_dma_scratch_size=40960)

    idx = nc.dram_tensor("idx", [128, idxcols], I16, kind="ExternalInput").ap()
    xT = nc.dram_tensor("xT", [cfg.F, PER], F32, kind="ExternalInput").ap()
    dinv_blk = nc.dram_tensor("dinv_blk", [P, NBLK], F32, kind="ExternalInput").ap()
    dinv2_blk = nc.dram_tensor("dinv2_blk", [P, NBLK], F32, kind="ExternalInput").ap()
    bias_blk = nc.dram_tensor("bias_blk", [P, NBLK, 16], F32, kind="ExternalInput").ap()
    mmats = nc.dram_tensor("mmats", [cfg.F, 48], F32, kind="ExternalInput").ap()
    m4 = nc.dram_tensor("m4", [cfg.F, 16], F32, kind="ExternalInput").ap()
    out = nc.dram_tensor("out", [P, NBLK, 16], F32, kind="ExternalOutput").ap()

    with tile.TileContext(nc) as tc:
        with (
            tc.tile_pool(name="const", bufs=1) as cp,
            tc.tile_pool(name="dram", bufs=1, space="DRAM") as dp,
        ):
            idx_sb = cp.tile([128, idxcols], I16)
            nc.sync.dma_start(idx_sb[:], idx)
            xT_sb = cp.tile([cfg.F, PER], F32)
            nc.sync.dma_start(xT_sb[:], xT)
            mm_sb = cp.tile([cfg.F, 48], F32)
            nc.sync.dma_start(mm_sb[:], mmats)
            m4_sb = cp.tile([cfg.F, 16], F32)
            nc.sync.dma_start(m4_sb[:], m4)
            db_sb = cp.tile([P, NBLK], F32)
            nc.sync.dma_start(db_sb[:], dinv_blk)
            d2_sb = cp.tile([P, NBLK], F32)
            nc.sync.dma_start(d2_sb[:], dinv2_blk)
            bias_sb = cp.tile([P, NBLK, 16], F32)
            nc.sync.dma_start(bias_sb[:], bias_blk)

            ttabs = [dp.tile([NPAD, cfg.F], F32, name=f"ttab{i}") for i in range(2)]
            # one ccin/ccout pair per exchange (4 exchanges x 2 halves)
            ccin = [[dp.tile([rows1 if h == 0 else rows2, 16], F32,
                             name=f"ccin{e}_{h}") for h in range(2)]
                    for e in range(4)]
            ccout = [[dp.tile([NC * (rows1 if h == 0 else rows2), 16], F32,
                              addr_space="Shared", name=f"ccout{e}_{h}")
                      for h in range(2)]
                     for e in range(4)]

            with (
                tc.tile_pool(name="gath", bufs=4) as gp,
                tc.tile_pool(name="work", bufs=3) as wp,
                tc.tile_pool(name="stage", bufs=2) as sp,
                tc.tile_pool(name="psum", bufs=2, space="PSUM") as psp,
            ):
                # Tile assigns Pool-DMA insts to DMASW lanes round-robin in
                # scheduled order; consumers wait on those lane sems.  The
                # descriptor-baked sem (sem=) must BE the matching lane sem,
                # and a global nosync chain pins scheduled order = emission
                # order so the rotation stays aligned.
                swdge_sems = tc.sems.swdge_block()
                n_lanes = len(swdge_sems)
                prep_count = [0]
                chain_tail = [None]

                def chain(binst):
                    if chain_tail[0] is not None:
                        deps = bass.InstructionNameOrderedSet()
                        deps.add(chain_tail[0])
                        binst.ins.add_nosync_dependencies_from(deps)
                    chain_tail[0] = binst.ins.name
                    return binst

                # unit index -> (lane sem, cumulative value at completion);
                # Tile's pass-2 wait generation wrongly elides DMA-completion
                # waits for prepared gathers (it models the DMASW tick as
                # firing at prep retirement), so consumers wait manually.
                unit_sem = {}

                def next_prep_sem(i):
                    c = prep_count[0]
                    prep_count[0] += 1
                    unit_sem[i] = (swdge_sems[c % n_lanes], 16 * (c // n_lanes + 1))
                    return swdge_sems[c % n_lanes]

                def make_stage():
                    st1 = sp.tile([P, nblk_h1, 16], F32, tag="stage1")
                    st2 = sp.tile([P, nblk_h2, 16], F32, tag="stage2")
                    return st1, st2

                def st_slot(sts, b):
                    return (sts[0][:, b, :] if b < nblk_h1
                            else sts[1][:, b - nblk_h1, :])

                def exchange(e, half, sts, target_tab):
                    ci = ccin[e][half]
                    co = ccout[e][half]
                    n_b = nblk_h1 if half == 0 else nblk_h2
                    base = 0 if half == 0 else nblk_h1
                    nc.scalar.dma_start(
                        ci[:].rearrange("(b p) f -> p b f", p=P), sts[half][:])
                    nc.gpsimd.collective_compute(
                        "AllGather", mybir.AluOpType.bypass,
                        replica_groups=[list(range(NC))],
                        ins=[ci[:]], outs=[co[:]],
                    )
                    # DRAM->DRAM restride: co rows (k, b, p) -> table rows
                    # k*PER + (base+b)*P + p, cols 0:16 of 64-wide rows
                    for k in range(NC):
                        dst = target_tab[k * PER + base * P:
                                         k * PER + (base + n_b) * P, 0:16]
                        nc.sync.dma_start(
                            dst, co[k * n_b * P:(k + 1) * n_b * P, :])


                # ---- gather units: (pass, group, window) ----
                units = []
                for p in range(4):
                    for g in range(cfg.NGRP):
                        units.append((p, g, 0))
                        if SBg[g]:
                            units.append((p, g, 1))
                nunits = len(units)
                gts = {}

                def emit_gather(i, prepare_only):
                    p, g, w = units[i]
                    q = i % cfg.NQ
                    sag, sbg = int(SAg[g]), int(SBg[g])
                    if w == 0:
                        gt_tile = gp.tile([P, sag + sbg, cfg.F], F32, tag="gt")
                        gts[(p, g)] = gt_tile
                    gt = gts[(p, g)]
                    tab = ttabs[p % 2][:]
                    if w == 0:
                        region = gt[:, 0:sag, :]
                        win = tab[0:cfg.WA_LEN, :]
                        cols = idx_sb[:, int(colA0[g]):int(colA0[g]) + sag * P // 16]
                        n_idx = sag * P
                    else:
                        region = gt[:, sag:sag + sbg, :]
                        win = tab[cfg.WB_OFF:NPAD, :]
                        cols = idx_sb[:, int(colB0[g]):int(colB0[g]) + sbg * P // 16]
                        n_idx = sbg * P
                    chain(nc.gpsimd.dma_gather(
                        out_ap=region,
                        in_ap=win,
                        idxs_ap=cols,
                        num_idxs=n_idx,
                        num_idxs_reg=n_idx,
                        elem_size=cfg.F,
                        single_packet=cfg.SINGLE_PACKET,
                        prepare_only=prepare_only,
                        sem=next_prep_sem(i) if prepare_only else None,
                        queue_num=q,
                    ))

                def emit_prep(i):
                    if cfg.PIPELINE:
                        emit_gather(i, True)

                def emit_trig(i):
                    if cfg.PIPELINE:
                        q = i % cfg.NQ
                        chain(nc.gpsimd.trigger_dma(count=None, queue_num=q))
                    else:
                        emit_gather(i, False)

                # ---- T3 stage: st = db * (x M4) per block, exchange to ttab0
                st_prev = make_stage()
                for bq in range(NBLK):
                    ps = psp.tile([P, 16], F32, tag="ps")
                    nc.tensor.matmul(out=ps[:], lhsT=xT_sb[:, bq * P:(bq + 1) * P],
                                     rhs=m4_sb[:], start=True, stop=True)
                    nc.scalar.activation(st_slot(st_prev, bq), ps[:], AF.Copy,
                                         scale=db_sb[:, bq:bq + 1])
                exchange(0, 0, st_prev, ttabs[0][:])
                exchange(0, 1, st_prev, ttabs[0][:])

                # Tile dependency tracking is emission-order based: a prep
                # reading table p must be emitted AFTER exchange p's table
                # writes, or the trigger never inherits the RAW dependency.
                prep_q = []
                allowed = [0]   # highest pass whose table writes are emitted

                def queue_prep(i):
                    prep_q.append(i)
                    while prep_q and units[prep_q[0]][0] <= allowed[0]:
                        emit_prep(prep_q.pop(0))

                for jj in range(min(cfg.LOOK, nunits)):
                    queue_prep(jj)

                st_cur = None
                for i in range(nunits):
                    p, g, w = units[i]
                    if g == 0 and w == 0:
                        # pass start: new stage generation
                        st_cur = make_stage()
                    emit_trig(i)
                    if i + cfg.LOOK < nunits:
                        queue_prep(i + cfg.LOOK)
                    last_w = 1 if SBg[g] else 0
                    if w != last_w:
                        continue
                    # ---- consume group g of pass p ----
                    gt = gts.pop((p, g))
                    sag = int(SAg[g])
                    if cfg.PIPELINE:
                        # manual DMA-completion waits (Tile elides them)
                        for ui in ([i - 1, i] if w == 1 else [i]):
                            sem_h, val = unit_sem.pop(ui)
                            nc.vector.wait_ge(sem_h, val)
                    for bq in groups[g]:
                        a0, a1 = int(oa[bq]), int(oa[bq] + SA[bq])
                        acc = wp.tile([P, 16], F32, tag="acc")
                        nc.vector.reduce_sum(
                            out=acc[:],
                            in_=gt[:, a0:a1, 0:16].rearrange("p s f -> p f s"),
                            axis=mybir.AxisListType.X,
                        )
                        if SB[bq]:
                            b0_ = sag + int(ob[bq])
                            b1_ = b0_ + int(SB[bq])
                            acc2 = wp.tile([P, 16], F32, tag="acc2")
                            nc.vector.reduce_sum(
                                out=acc2[:],
                                in_=gt[:, b0_:b1_, 0:16].rearrange("p s f -> p f s"),
                                axis=mybir.AxisListType.X,
                            )
                            nc.vector.tensor_add(out=acc[:], in0=acc[:], in1=acc2[:])
                        # R_full = R + self message (previous stage value)
                        t1 = wp.tile([P, 16], F32, tag="t1")
                        nc.vector.tensor_add(out=t1[:], in0=acc[:],
                                             in1=st_slot(st_prev, bq))
                        if p < 3:
                            ps = psp.tile([P, 16], F32, tag="ps")
                            nc.tensor.matmul(
                                out=ps[:], lhsT=xT_sb[:, bq * P:(bq + 1) * P],
                                rhs=mm_sb[:, 16 * p:16 * p + 16],
                                start=True, stop=True)
                            ta = wp.tile([P, 16], F32, tag="ta")
                            nc.scalar.activation(ta[:], ps[:], AF.Copy,
                                                 scale=db_sb[:, bq:bq + 1])
                            tb = wp.tile([P, 16], F32, tag="tb")
                            nc.scalar.activation(tb[:], t1[:], AF.Copy,
                                                 scale=d2_sb[:, bq:bq + 1])
                            nc.vector.tensor_add(out=st_slot(st_cur, bq),
                                                 in0=ta[:], in1=tb[:])
                        else:
                            tb = wp.tile([P, 16], F32, tag="tb")
                            nc.scalar.activation(tb[:], t1[:], AF.Copy,
                                                 scale=db_sb[:, bq:bq + 1])
                            nc.vector.tensor_add(out=st_slot(st_cur, bq),
                                                 in0=tb[:], in1=bias_sb[:, bq, :])
                    if p < 3 and g == cfg.H1GRP:
                        # half-1 data (groups 0..H1GRP-1) is ready one group
                        # earlier; emitting here keeps the collective's input
                        # wait off the Pool engine's critical path
                        exchange(p + 1, 0, st_cur, ttabs[(p + 1) % 2][:])
                    if p < 3 and g == cfg.NGRP - 1:
                        exchange(p + 1, 1, st_cur, ttabs[(p + 1) % 2][:])
                        allowed[0] = p + 1
                        while prep_q and units[prep_q[0]][0] <= allowed[0]:
                            emit_prep(prep_q.pop(0))
                    if p == 3 and g == cfg.NGRP - 1:
                        nc.sync.dma_start(out[:, 0:nblk_h1, :], st_cur[0][:])
                        nc.sync.dma_start(out[:, nblk_h1:NBLK, :], st_cur[1][:])
                    if p < 3 and g == cfg.NGRP - 1:
                        st_prev = st_cur

    return nc


# --------------------------------------------------------------------------
# entry point
# --------------------------------------------------------------------------

def _run(inputs, cfg: Cfg, runner=None, **run_kwargs):
    """runner(nc, in_maps) -> list[dict] allows sim injection for testing."""
    global LAST_RESULTS
    in_maps, layout, old2new = _host_prep(inputs, cfg)
    nc = _build_module(cfg, layout)
    nc.compile()
    if runner is None:
        res = run_bass_kernel_spmd(nc, in_maps, core_ids=list(range(cfg.NCORES)),
                                   **run_kwargs)
        LAST_RESULTS = res
        outs = res.results
    else:
        outs = runner(nc, in_maps)
    full = np.empty((cfg.NPAD, 16), np.float32)
    for k in range(cfg.NCORES):
        o = np.asarray(outs[k]["out"])  # [P, NBLK, 16]
        full[k * cfg.PER:(k + 1) * cfg.PER] = o.transpose(1, 0, 2).reshape(cfg.PER, 16)
    return full[old2new]


def kernel(**inputs) -> np.ndarray:
    return _run(inputs, CFG)

